# revision 22
# baseline (speedup 1.0000x reference)
"""Trainium2 Bass kernel for ASH1DSelfAttention (sparse attention).

Strategy (8 cores, SPMD, core-agnostic program; all per-core variation enters
via input tensors):
  - core c handles batch b = c//4, query slice ts = [256*(c%4), +256), all 8 heads.
  - Sparse softmax over 80 data-dependent candidates per query is reformulated
    densely over all 1024 key positions:
        U[t,j] = exp(Wd[t,j] * dot[t,j]) - 1 + Md[t,j]
        out[t] = (U @ V)[t] / sum_j U[t,j]
    where Wd scatters the (dup/causal-masked, normalized) gaussian mixture
    weights of the first occurrence of each candidate index, and Md scatters
    its multiplicity.  Non-candidates contribute exp(0)-1+0 = 0; duplicate
    candidates contribute their extra exp(0)=1 via Md.
  - Duplicate detection / multiplicity: pairwise-equality of the 80 candidate
    indices via two big strided tensor ops (all shift distances d=1..79 in one
    AP) + log-tree reductions -> cntL (earlier-equal count) / cntR.
  - Scatters: GPSIMD local_scatter (per-partition indices, fp16, dup-free by
    masking non-first occurrences to index -1).
  - Dense phase is done transposed (j on partitions) so the U @ V contraction
    runs directly on the PE without transposing U:  per j-chunk, dot^T is
    computed per head, logits = Wd^T (*) dot^T, E = exp(logits), then
    out^T[h] (65 x 256, last row = Z via the ones-column of V_aug) accumulates
    lhsT=V_aug chunk against rhs {E^T chunk, (Md^T - 1) chunk}.
"""

import os
import numpy as np

B, T, E, H, K = 2, 1024, 64, 8, 8
GADD, RADD, REGION = 4, 4, 64
VS = K * (2 + GADD + RADD)  # 80
TS = 256  # queries per core
NC = 8

_prog_cache = {}


def _host_constants():
    """gidx/roff depend only on the fixed key 42 -> host-precomputable."""
    import jax
    with jax.default_device(jax.devices("cpu")[0]):
        kg, kr = jax.random.split(jax.random.key(42))
        gidx = np.asarray(jax.random.randint(kg, (B, T, K, GADD), 0, T))
        roff = np.asarray(jax.random.randint(kr, (B, T, K, RADD), 0, REGION))
    return gidx, roff


def build_program():
    import concourse.bass as bass
    import concourse.bacc as bacc
    import concourse.tile as tile
    import concourse.mybir as mybir
    from contextlib import ExitStack

    dt = mybir.dt
    AF = mybir.ActivationFunctionType
    OP = mybir.AluOpType
    AX = mybir.AxisListType
    f32, f16, i16, i32 = dt.float32, dt.float16, dt.int16, dt.int32

    nc = bacc.Bacc("TRN2", target_bir_lowering=False, debug=False)

    def din(name, shape, dtyp=f32):
        return nc.dram_tensor(name, shape, dtyp, kind="ExternalInput").ap()

    xbT = din("xbT", (64, 1024))
    xsT = din("xsT", (64, TS))
    inpT = din("inpT", (65, TS))
    W1Ta = din("W1Ta", (65, 128))
    W1Tb = din("W1Tb", (65, 128))
    bp1c = din("bp1c", (256, 1))
    W2Ta = din("W2Ta", (128, 16))
    W2Tb = din("W2Tb", (128, 16))
    bp2c = din("bp2c", (16, 1))
    WkTs = din("WkTs", (64, 512))
    WqT = din("WqT", (64, 512))
    WvT = din("WvT", (64, 512))
    WuT = din("WuT", (512, 64))
    buR = din("buR", (1, 64))
    tposP = din("tposP", (TS, 1))
    gidxF = din("gidxF", (TS, 32))
    roffM = din("roffM", (TS, 32))
    Ssel = din("Ssel", (32, 128))
    DBG = bool(int(os.environ.get("BASS_KERNEL_DEBUG", "0")))
    idF16 = din("idF16", (128, 128), f16)
    idF32 = din("idF32", (128, 128), f32)
    outD = nc.dram_tensor("out", (TS, 64), f32, kind="ExternalOutput").ap()
    dbg = {}
    if DBG:
        def dout(name, shape, dtyp):
            dbg[name] = nc.dram_tensor(name, shape, dtyp, kind="ExternalOutput").ap()
        dout("d_idx", (TS, 80), f32)
        dout("d_cntL", (TS, 80), f16)
        dout("d_cntR", (TS, 80), f16)
        dout("d_wts", (TS, 80), f32)
        dout("d_wd", (TS, 1024), f16)
        dout("d_md", (TS, 1024), f16)
        dout("d_dt", (128, 2048), f16)
        dout("d_eb", (128, 2048), f32)
        dout("d_psO", (65, 2048), f32)
        dout("d_invZ", (8, TS), f32)

    def vap(t, dims, off=0):
        a = t[:] if not isinstance(t, bass.AP) else t
        return bass.AP(tensor=a.tensor, offset=a.offset + off,
                       ap=[list(a.ap[0])] + [list(d) for d in dims])

    with tile.TileContext(nc) as tc, ExitStack() as ctx:
        P = ctx.enter_context(tc.tile_pool(name="persist", bufs=1))
        WK = ctx.enter_context(tc.tile_pool(name="work", bufs=2))
        W1 = ctx.enter_context(tc.tile_pool(name="work1", bufs=1))
        PS = ctx.enter_context(tc.tile_pool(name="psum", bufs=2, space="PSUM"))
        PSO = ctx.enter_context(tc.tile_pool(name="psumO", bufs=1, space="PSUM"))

        # ---------------- const loads ----------------
        def load(name, ap_in, shape, dtyp=f32, tag=None):
            t = P.tile(list(shape), dtyp, tag=tag or name)
            nc.sync.dma_start(out=t[:], in_=ap_in)
            return t

        xbT_sb = load("xbT_sb", xbT, (64, 1024))
        xsT_sb = load("xsT_sb", xsT, (64, TS))
        inpT_sb = load("inpT_sb", inpT, (65, TS))
        w1a = load("w1a", W1Ta, (65, 128))
        w1b = load("w1b", W1Tb, (65, 128))
        bp1a = load("bp1a", bp1c[0:128, :], (128, 1))
        bp1b = load("bp1b", bp1c[128:256, :], (128, 1))
        w2a = load("w2a", W2Ta, (128, 16))
        w2b = load("w2b", W2Tb, (128, 16))
        bp2_sb = load("bp2_sb", bp2c, (16, 1))
        wk_sb = load("wk_sb", WkTs, (64, 512))
        wq_sb = load("wq_sb", WqT, (64, 512))
        wv_sb = load("wv_sb", WvT, (64, 512))
        wu_sb = [load(f"wu{pc}", WuT[pc * 128:(pc + 1) * 128, :], (128, 64))
                 for pc in range(4)]
        bu_sb = load("bu_sb", buR, (1, 64))
        tp_sb = [load(f"tp{tt}", tposP[tt * 128:(tt + 1) * 128, :], (128, 1))
                 for tt in range(2)]
        gi_sb = [load(f"gi{tt}", gidxF[tt * 128:(tt + 1) * 128, :], (128, 32))
                 for tt in range(2)]
        ro_sb = [load(f"ro{tt}", roffM[tt * 128:(tt + 1) * 128, :], (128, 32))
                 for tt in range(2)]
        ssel_sb = [load(f"ssel{hp}", Ssel[hp * 8:(hp + 1) * 8, :], (8, 128))
                   for hp in range(4)]
        idf16 = load("idf16", idF16, (128, 128), f16)
        idf32 = load("idf32", idF32, (128, 128), f32)
        c1023 = P.tile([128, 1], f32, tag="c1023", name="c1023")
        nc.vector.memset(c1023[:], 1023.0)
        ones1 = P.tile([1, 128], f32, tag="ones1", name="ones1")
        nc.vector.memset(ones1[:], 1.0)
        c1 = P.tile([128, 1], f32, tag="c1", name="c1")
        nc.vector.memset(c1[:], 1.0)
        c2 = P.tile([128, 1], f32, tag="c2", name="c2")
        nc.vector.memset(c2[:], 2.0)
        cm1 = P.tile([128, 1], f32, tag="cm1", name="cm1")
        nc.vector.memset(cm1[:], -1.0)
        zl = P.tile([1, 65], f32, tag="zl", name="zl")
        nc.vector.memset(zl[:], 0.0)
        zr = P.tile([1, 512], f32, tag="zr", name="zr")
        nc.vector.memset(zr[:], 0.0)

        # ---------------- projections ----------------
        # K^T (512,1024) as 4 chunks (128,1024); rows h*64+e, scaled 1/8 via WkTs
        kt = [P.tile([128, 1024], f32, tag=f"kt{mc}", name=f"kt{mc}") for mc in range(4)]
        qt = [P.tile([128, TS], f32, tag=f"qt{mc}", name=f"qt{mc}") for mc in range(4)]
        for mc in range(4):
            for half in range(2):
                pk = PS.tile([128, 512], f32, tag="ps", name="pk")
                nc.tensor.matmul(pk[:], wk_sb[:, mc * 128:(mc + 1) * 128],
                                 xbT_sb[:, half * 512:(half + 1) * 512],
                                 start=True, stop=True)
                nc.scalar.copy(kt[mc][:, half * 512:(half + 1) * 512], pk[:])
            pq = PS.tile([128, TS], f32, tag="ps", name="pq")
            nc.tensor.matmul(pq[:], wq_sb[:, mc * 128:(mc + 1) * 128], xsT_sb[:],
                             start=True, stop=True)
            nc.scalar.copy(qt[mc][:], pq[:])

        # V_aug (j,520) f32 per j-chunk: cols h*65+[0..63] = V, col h*65+64 = 1
        vaug = [P.tile([128, 520], f32, tag=f"va{jc}", name=f"va{jc}") for jc in range(8)]
        for jc in range(8):
            pv = PS.tile([128, 512], f32, tag="ps", name="pv")
            nc.tensor.matmul(pv[:], xbT_sb[:, jc * 128:(jc + 1) * 128], wv_sb[:],
                             start=True, stop=True)
            va3 = vaug[jc].rearrange("p (h c) -> p h c", h=8)
            nc.scalar.copy(va3[:, :, 0:64], pv[:].rearrange("p (h c) -> p h c", h=8))
            nc.vector.memset(va3[:, :, 64:65], 1.0)

        # ---------------- hypernet ----------------
        r1 = [P.tile([128, TS], f32, tag=f"r1{i}", name=f"r1{i}") for i in range(2)]
        for i, (w1, bp1_) in enumerate([(w1a, bp1a), (w1b, bp1b)]):
            p1 = PS.tile([128, TS], f32, tag="ps", name="p1")
            nc.tensor.matmul(p1[:], w1[:], inpT_sb[:], start=True, stop=True)
            nc.scalar.activation(r1[i][:], p1[:], AF.Relu, bias=bp1_[:])
        p2 = PS.tile([16, TS], f32, tag="ps", name="p2")
        nc.tensor.matmul(p2[:], w2a[:], r1[0][:], start=True, stop=False)
        nc.tensor.matmul(p2[:], w2b[:], r1[1][:], start=False, stop=True)
        p2T_sb = P.tile([16, TS], f32, tag="p2T", name="p2T")
        nc.scalar.activation(p2T_sb[:], p2[:], AF.Identity, bias=bp2_sb[:])
        p2_sb = [P.tile([128, 16], f32, tag=f"p2_{tt}", name=f"p2_{tt}") for tt in range(2)]
        for tt in range(2):
            pt = PS.tile([128, 16], f32, tag="ps", name="pt")
            nc.tensor.transpose(pt[:], p2T_sb[:, tt * 128:(tt + 1) * 128],
                                idf32[0:16, 0:16])
            nc.scalar.copy(p2_sb[tt][:], pt[:])

        # ---------------- per t-tile sparse stage ----------------
        wd = [P.tile([128, 1024], f16, tag=f"wd{tt}", name=f"wd{tt}") for tt in range(2)]
        md = [P.tile([128, 1024], f16, tag=f"md{tt}", name=f"md{tt}") for tt in range(2)]
        for tt in range(2):
            tp = tp_sb[tt]
            # means / sigmas
            sp = WK.tile([128, 8], f32, tag="sp", name="sp")
            nc.scalar.activation(sp[:], p2_sb[tt][:, 0:8], AF.Exp)
            nc.scalar.activation(sp[:], sp[:], AF.Ln, bias=c1[:])
            means = WK.tile([128, 8], f32, tag="means", name="means")
            nc.vector.scalar_tensor_tensor(means[:], sp[:], -3.0,
                                           vap(tp, [[0, 8]]), OP.mult, OP.add)
            nc.vector.scalar_tensor_tensor(means[:], means[:], 0.0,
                                           vap(c1023, [[0, 8]]), OP.max, OP.min)
            fli = WK.tile([128, 8], i32, tag="fli", name="fli")
            nc.vector.tensor_copy(fli[:], means[:])
            flr = WK.tile([128, 8], f32, tag="flr", name="flr")
            nc.vector.tensor_copy(flr[:], fli[:])
            # robust floor under either cast-rounding mode: r - (r > x)
            fgt = WK.tile([128, 8], f32, tag="fgt", name="fgt")
            nc.vector.tensor_tensor(fgt[:], flr[:], means[:], OP.is_gt)
            fl = WK.tile([128, 8], f32, tag="fl", name="fl")
            nc.vector.tensor_tensor(fl[:], flr[:], fgt[:], OP.subtract)
            sp2 = WK.tile([128, 8], f32, tag="sp2", name="sp2")
            nc.scalar.activation(sp2[:], p2_sb[tt][:, 8:16], AF.Exp, bias=c2[:])
            nc.scalar.activation(sp2[:], sp2[:], AF.Ln, bias=c1[:])
            sig = WK.tile([128, 8], f32, tag="sig", name="sig")
            nc.vector.tensor_scalar(sig[:], sp2[:], 0.05, 102.4, OP.add, OP.mult)
            invsig = WK.tile([128, 8], f32, tag="invsig", name="invsig")
            nc.vector.reciprocal(invsig[:], sig[:])

            # candidate indices (128, 80) = 8 groups of [n0 n1 g g g g r r r r]
            idxF = WK.tile([128, 8, 10], f32, tag="idxF", name="idxF")
            nc.vector.tensor_copy(idxF[:, :, 0:1], vap(fl, [[1, 8], [0, 1]]))
            nc.vector.scalar_tensor_tensor(idxF[:, :, 1:2], vap(fl, [[1, 8], [0, 1]]), 1.0,
                                           vap(c1023, [[0, 8], [0, 1]]), OP.add, OP.min)
            nc.vector.tensor_copy(idxF[:, :, 2:6],
                                  gi_sb[tt][:].rearrange("p (k g) -> p k g", k=8))
            rtmp = WK.tile([128, 8, 4], f32, tag="rtmp", name="rtmp")
            nc.vector.tensor_tensor(rtmp[:], ro_sb[tt][:].rearrange("p (k g) -> p k g", k=8),
                                    vap(fl, [[1, 8], [0, 4]]), OP.add)
            nc.vector.scalar_tensor_tensor(idxF[:, :, 6:10], rtmp[:], 0.0,
                                           vap(c1023, [[0, 8], [0, 4]]), OP.max, OP.min)
            idxf = idxF[:].rearrange("p k g -> p (k g)")

            causal = WK.tile([128, VS], f32, tag="causal", name="causal")
            nc.vector.tensor_scalar(causal[:], idxf, tp[:], None, OP.is_gt)

            # pairwise-equality counts cntL / cntR over shift distances 1..79
            idxA = W1.tile([128, 160], f16, tag="idxA", name="idxA")
            nc.vector.memset(idxA[:], -2.0)
            nc.vector.tensor_copy(idxA[:, 0:80], idxf)
            idxB = W1.tile([128, 240], f16, tag="idxB", name="idxB")
            nc.vector.memset(idxB[:, 0:80], -1.0)
            nc.vector.tensor_copy(idxB[:, 80:160], idxf)
            nc.vector.memset(idxB[:, 160:240], -3.0)
            idxBo = W1.tile([128, 240], f16, tag="idxBo", name="idxBo")
            nc.vector.memset(idxBo[:, 239:240], -3.0)
            nc.vector.tensor_copy(idxBo[:, 0:239], idxB[:, 1:240])

            cbuf = W1.tile([128, 80 * 160], f16, tag="cbuf", name="cbuf")
            nc.vector.memset(cbuf[:, 0:160], 0.0)
            # even d = 2,4,..,78 (39 planes): in1 row start 80-d (even)
            nc.vector.tensor_tensor(
                vap(cbuf, [[320, 39], [1, 160]], off=320),
                vap(idxA, [[0, 39], [1, 160]]),
                vap(idxB, [[-2, 39], [1, 160]], off=78), OP.is_equal)
            # odd d = 1,3,..,79 (40 planes): idxBo[i] = idxB[i+1], start 80-d-1 (even)
            nc.vector.tensor_tensor(
                vap(cbuf, [[320, 40], [1, 160]], off=160),
                vap(idxA, [[0, 40], [1, 160]]),
                vap(idxBo, [[-2, 40], [1, 160]], off=78), OP.is_equal)

            def tree(first_level_in0, first_level_in1, tag):
                lv = W1.tile([128, 40 * 80], f16, tag=f"{tag}40", name=f"{tag}40")
                nc.vector.tensor_tensor(lv[:], first_level_in0, first_level_in1, OP.add)
                sizes = [20, 10, 5]
                for s in sizes:
                    nxt = W1.tile([128, s * 80], f16, tag=f"{tag}{s}", name=f"{tag}{s}")
                    nc.vector.tensor_tensor(
                        nxt[:], vap(lv, [[160, s], [1, 80]]),
                        vap(lv, [[160, s], [1, 80]], off=80), OP.add)
                    lv = nxt
                # 5 planes -> (0+1)+(2+3) then +4
                s2 = W1.tile([128, 2 * 80], f16, tag=f"{tag}2", name=f"{tag}2")
                nc.vector.tensor_tensor(s2[:], vap(lv, [[160, 2], [1, 80]]),
                                        vap(lv, [[160, 2], [1, 80]], off=80), OP.add)
                s1 = WK.tile([128, 80], f16, tag=f"{tag}1", name=f"{tag}1")
                nc.vector.tensor_tensor(s1[:], s2[:, 0:80], s2[:, 80:160], OP.add)
                cnt = WK.tile([128, 80], f16, tag=f"{tag}c", name=f"{tag}c")
                nc.vector.tensor_tensor(cnt[:], s1[:], vap(lv, [[1, 80]], off=320),
                                        OP.add)
                return cnt

            # L: plane d at offset 160*d ; pairs (2e, 2e+1), d=0 plane is zero
            cntL = tree(vap(cbuf, [[320, 40], [1, 80]]),
                        vap(cbuf, [[320, 40], [1, 80]], off=160), "tl")
            # R: plane d at offset 161*d (reads into -2 guard zone give 0)
            cntR = tree(vap(cbuf, [[322, 40], [1, 80]]),
                        vap(cbuf, [[322, 40], [1, 80]], off=161), "tr")

            firstf = WK.tile([128, VS], f32, tag="firstf", name="firstf")
            nc.vector.tensor_scalar(firstf[:], cntL[:], 0.0, None, OP.is_equal)
            pmask = WK.tile([128, VS], f32, tag="pmask", name="pmask")
            nc.vector.tensor_tensor(pmask[:], firstf[:], causal[:], OP.is_gt)

            # props (v-major, k-minor) and mixture weights
            zd = W1.tile([128, VS, 8], f32, tag="zd", name="zd")
            nc.vector.tensor_tensor(zd[:], vap(idxF, [[1, 80], [0, 8]]),
                                    vap(means, [[0, 80], [1, 8]]), OP.subtract)
            nc.vector.tensor_tensor(zd[:], zd[:], vap(invsig, [[0, 80], [1, 8]]),
                                    OP.mult)
            sq = W1.tile([128, VS, 8], f32, tag="sq", name="sq")
            nc.scalar.activation(sq[:], zd[:], AF.Square)
            props = W1.tile([128, VS, 8], f32, tag="props", name="props")
            nc.scalar.activation(props[:], sq[:], AF.Exp, scale=-0.5)
            nc.vector.tensor_tensor(props[:], props[:], vap(pmask, [[1, 80], [0, 8]]),
                                    OP.mult)
            ssum = WK.tile([128, 8], f32, tag="ssum", name="ssum")
            nc.vector.tensor_reduce(ssum[:], vap(props, [[1, 8], [8, 80]]),
                                    AX.X, OP.add)
            invS = WK.tile([128, 8], f32, tag="invS", name="invS")
            nc.vector.reciprocal(invS[:], ssum[:])
            nc.vector.tensor_tensor(props[:], props[:], vap(invS, [[0, 80], [1, 8]]),
                                    OP.mult)
            wts = WK.tile([128, VS], f32, tag="wts", name="wts")
            nc.vector.tensor_reduce(wts[:], props[:], AX.X, OP.add)

            # scatter data / masked indices
            w16 = WK.tile([128, VS], f16, tag="w16", name="w16")
            nc.vector.tensor_copy(w16[:], wts[:])
            mult16 = WK.tile([128, VS], f16, tag="mult16", name="mult16")
            nc.vector.tensor_tensor(mult16[:], cntL[:], cntR[:], OP.add)
            nc.vector.tensor_scalar(mult16[:], mult16[:], 1.0, None, OP.add)
            idm = WK.tile([128, VS], f32, tag="idm", name="idm")
            nc.vector.scalar_tensor_tensor(idm[:], idxf, 1.0, firstf[:],
                                           OP.add, OP.mult)
            nc.vector.tensor_scalar(idm[:], idm[:], -1.0, None, OP.add)
            idm16 = WK.tile([128, VS], i16, tag="idm16", name="idm16")
            nc.vector.tensor_copy(idm16[:], idm[:])

            nc.gpsimd.local_scatter(wd[tt][:], w16[:], idm16[:], 128, 1024, VS)
            nc.gpsimd.local_scatter(md[tt][:], mult16[:], idm16[:], 128, 1024, VS)
            if DBG:
                sl = slice(tt * 128, (tt + 1) * 128)
                nc.sync.dma_start(out=dbg["d_idx"][sl, :], in_=idxf)
                nc.sync.dma_start(out=dbg["d_cntL"][sl, :], in_=cntL[:])
                nc.sync.dma_start(out=dbg["d_cntR"][sl, :], in_=cntR[:])
                nc.sync.dma_start(out=dbg["d_wts"][sl, :], in_=wts[:])
                nc.sync.dma_start(out=dbg["d_wd"][sl, :], in_=wd[tt][:])
                nc.sync.dma_start(out=dbg["d_md"][sl, :], in_=md[tt][:])

        # ---------------- transposes of Wd / Md ----------------
        wdT = [P.tile([128, TS], f16, tag=f"wdT{jc}", name=f"wdT{jc}") for jc in range(8)]
        mm1T = [P.tile([128, TS], f32, tag=f"mm1T{jc}", name=f"mm1T{jc}") for jc in range(8)]
        for jc in range(8):
            for tt in range(2):
                ptw = PS.tile([128, 128], f16, tag="pst", name="ptw")
                nc.tensor.transpose(ptw[:], wd[tt][:, jc * 128:(jc + 1) * 128],
                                    idf16[:])
                nc.scalar.copy(wdT[jc][:, tt * 128:(tt + 1) * 128], ptw[:])
                ptm = PS.tile([128, 128], f16, tag="pst", name="ptm")
                nc.tensor.transpose(ptm[:], md[tt][:, jc * 128:(jc + 1) * 128],
                                    idf16[:])
                nc.scalar.activation(mm1T[jc][:, tt * 128:(tt + 1) * 128], ptm[:],
                                     AF.Identity, bias=cm1[:])

        # ---------------- dense phase: per j-chunk ----------------
        psO = PSO.tile([65, 8, TS], f32, tag="psO", name="psO")
        # each start=True clears its whole PSUM bank, so pre-clear each of the
        # 4 banks once (k=1 zero matmul over the full 512-f32 bank span) and
        # accumulate everything else with start=False.
        for m in range(4):
            nc.tensor.matmul(vap(psO, [[1, 512]], off=512 * m), zl[:], zr[:],
                             start=True, stop=False, skip_group_check=True)
        for jc in range(8):
            dt16 = WK.tile([128, 8, TS], f16, tag="dt16", name="dt16")
            for h in range(8):
                pd = PS.tile([128, TS], f32, tag="ps", name="pd")
                nc.tensor.matmul(
                    pd[:],
                    kt[h // 2][(h % 2) * 64:(h % 2) * 64 + 64, jc * 128:(jc + 1) * 128],
                    qt[h // 2][(h % 2) * 64:(h % 2) * 64 + 64, :],
                    start=True, stop=True)
                nc.scalar.copy(dt16[:, h:h+1, :], vap(pd, [[0, 1], [1, TS]]))
            lg16 = WK.tile([128, 8, TS], f16, tag="lg16", name="lg16")
            nc.vector.tensor_tensor(lg16[:], dt16[:],
                                    vap(wdT[jc], [[0, 8], [1, TS]]), OP.mult)
            ebig = WK.tile([128, 8, TS], f32, tag="ebig", name="ebig")
            nc.scalar.activation(ebig[:], lg16[:], AF.Exp)
            if DBG and jc == 0:
                nc.sync.dma_start(out=dbg["d_dt"][:], in_=dt16[:])
                nc.sync.dma_start(out=dbg["d_eb"][:], in_=ebig[:])
            for h in range(8):
                va = vaug[jc][:, h * 65:h * 65 + 65]
                nc.tensor.matmul(psO[:, h, :], va, ebig[:, h, :],
                                 start=False, stop=False, skip_group_check=True)
                nc.tensor.matmul(psO[:, h, :], va, mm1T[jc][:],
                                 start=False, stop=(jc == 7), skip_group_check=True)

        # ---------------- normalize + output proj ----------------
        if DBG:
            dps = P.tile([65, 8, TS], f32, tag="dps", name="dps")
            nc.scalar.copy(dps[:], psO[:])
            nc.sync.dma_start(out=dbg["d_psO"][:], in_=dps[:].rearrange("p a b -> p (a b)"))
        invZf = P.tile([1, 8, TS], f32, tag="invZf", name="invZf")
        nc.vector.reciprocal(invZf[:], psO[64:65, :, :])
        invZ = P.tile([8, TS], f32, tag="invZ", name="invZ")
        nc.sync.dma_start(out=invZ[:], in_=invZf[:])
        if DBG:
            nc.sync.dma_start(out=dbg["d_invZ"][:], in_=invZ[:])
        packed = [P.tile([128, TS], f32, tag=f"packed{pc}", name=f"packed{pc}") for pc in range(4)]
        for hp in range(4):
            pb = PS.tile([128, TS], f32, tag="ps", name="pb")
            nc.tensor.matmul(pb[:], ssel_sb[hp][:], invZ[:],
                             start=True, stop=True)
            bc = WK.tile([128, TS], f32, tag="bc", name="bc")
            nc.scalar.copy(bc[:], pb[:])
            for i in range(2):
                h = 2 * hp + i
                nc.vector.tensor_tensor(packed[hp][i * 64:(i + 1) * 64, :],
                                        psO[0:64, h, :], bc[i * 64:(i + 1) * 64, :],
                                        OP.mult)
        for th in range(2):
            pf = PS.tile([128, 64], f32, tag="ps", name="pf")
            for pc in range(4):
                nc.tensor.matmul(pf[:], packed[pc][:, th * 128:(th + 1) * 128],
                                 wu_sb[pc][:], start=(pc == 0), stop=False)
            nc.tensor.matmul(pf[:], ones1[:], bu_sb[:],
                             start=False, stop=True)
            ofin = WK.tile([128, 64], f32, tag="ofin", name="ofin")
            nc.scalar.copy(ofin[:], pf[:])
            nc.sync.dma_start(out=outD[th * 128:(th + 1) * 128, :], in_=ofin[:])

    nc.compile()
    return nc


def _make_in_maps(inputs):
    x = np.asarray(inputs["x"], np.float32)
    Wk = np.asarray(inputs["Wk"], np.float32)
    Wq = np.asarray(inputs["Wq"], np.float32)
    Wv = np.asarray(inputs["Wv"], np.float32)
    Wu = np.asarray(inputs["Wu"], np.float32)
    bu = np.asarray(inputs["bu"], np.float32)
    Wp1 = np.asarray(inputs["Wp1"], np.float32)
    bp1 = np.asarray(inputs["bp1"], np.float32)
    Wp2 = np.asarray(inputs["Wp2"], np.float32)
    bp2 = np.asarray(inputs["bp2"], np.float32)

    gidx, roff = _host_constants()
    tpos = np.arange(T, dtype=np.float32)
    C = np.ascontiguousarray

    ssel = np.zeros((32, 128), np.float32)
    for hp in range(4):
        for r in range(8):
            for p in range(128):
                if r == 2 * hp + p // 64:
                    ssel[hp * 8 + r, p] = 1.0
    ident16 = np.eye(128, dtype=np.float16)

    shared = {
        "W1Ta": C(Wp1[0:128].T), "W1Tb": C(Wp1[128:256].T),
        "bp1c": C(bp1.reshape(256, 1)),
        "W2Ta": C(Wp2[:, 0:128].T), "W2Tb": C(Wp2[:, 128:256].T),
        "bp2c": C(bp2.reshape(16, 1)),
        "WkTs": C(Wk.T / 8.0), "WqT": C(Wq.T), "WvT": C(Wv.T),
        "WuT": C(Wu.T), "buR": C(bu.reshape(1, 64)),
        "Ssel": ssel, "idF16": ident16,
        "idF32": np.eye(128, dtype=np.float32),
    }
    in_maps = []
    for c in range(NC):
        b, t0 = c // 4, TS * (c % 4)
        ts = slice(t0, t0 + TS)
        m = dict(shared)
        m["xbT"] = C(x[b].T)
        m["xsT"] = C(x[b, ts].T)
        m["inpT"] = C(np.vstack([x[b, ts].T, (tpos[ts] / T)[None, :]]))
        m["tposP"] = C(tpos[ts].reshape(TS, 1))
        m["gidxF"] = C(gidx[b, ts].reshape(TS, 32).astype(np.float32))
        m["roffM"] = C((roff[b, ts].reshape(TS, 32) - REGION // 2).astype(np.float32))
        in_maps.append(m)
    return in_maps


def kernel(**inputs):
    from concourse import bass_utils
    if "nc" not in _prog_cache:
        _prog_cache["nc"] = build_program()
    nc = _prog_cache["nc"]
    in_maps = _make_in_maps(inputs)
    res = bass_utils.run_bass_kernel_spmd(
        nc, in_maps, core_ids=list(range(NC)),
        trace=bool(int(os.environ.get("BASS_KERNEL_TRACE", "0"))))
    _prog_cache["last_result"] = res
    out = np.zeros((B, T, E), np.float32)
    for c in range(NC):
        b, t0 = c // 4, TS * (c % 4)
        out[b, t0:t0 + TS] = res.results[c]["out"]
    return out


# revision 45
# speedup vs baseline: 1.0029x; 1.0029x over previous
"""Trainium2 Bass kernel for ASH1DSelfAttention (sparse attention).

Strategy (8 cores, SPMD, core-agnostic program; all per-core variation enters
via input tensors):
  - core c handles batch b = c//4, query slice ts = [256*(c%4), +256), all 8 heads.
  - Sparse softmax over 80 data-dependent candidates per query is reformulated
    densely over all 1024 key positions:
        U[t,j] = exp(Wd[t,j] * dot[t,j]) - 1 + Md[t,j]
        out[t] = (U @ V)[t] / sum_j U[t,j]
    where Wd scatters the (dup/causal-masked, normalized) gaussian mixture
    weights of the first occurrence of each candidate index, and Md scatters
    its multiplicity.  Non-candidates contribute exp(0)-1+0 = 0; duplicate
    candidates contribute their extra exp(0)=1 via Md.
  - Duplicate detection / multiplicity: pairwise-equality of the 80 candidate
    indices via two big strided tensor ops (all shift distances d=1..79 in one
    AP) + log-tree reductions -> cntL (earlier-equal count) / cntR.
  - Scatters: GPSIMD local_scatter (per-partition indices, fp16, dup-free by
    masking non-first occurrences to index -1).
  - Dense phase is done transposed (j on partitions) so the U @ V contraction
    runs directly on the PE without transposing U:  per j-chunk, dot^T is
    computed per head, logits = Wd^T (*) dot^T, E = exp(logits), then
    out^T[h] (65 x 256, last row = Z via the ones-column of V_aug) accumulates
    lhsT=V_aug chunk against rhs {E^T chunk, (Md^T - 1) chunk}.
"""

import os
import numpy as np

B, T, E, H, K = 2, 1024, 64, 8, 8
GADD, RADD, REGION = 4, 4, 64
VS = K * (2 + GADD + RADD)  # 80
TS = 256  # queries per core
NC = 8

_prog_cache = {}


def _host_constants():
    """gidx/roff depend only on the fixed key 42 -> host-precomputable."""
    import jax
    with jax.default_device(jax.devices("cpu")[0]):
        kg, kr = jax.random.split(jax.random.key(42))
        gidx = np.asarray(jax.random.randint(kg, (B, T, K, GADD), 0, T))
        roff = np.asarray(jax.random.randint(kr, (B, T, K, RADD), 0, REGION))
    return gidx, roff


def build_program():
    import concourse.bass as bass
    import concourse.bacc as bacc
    import concourse.tile as tile
    import concourse.mybir as mybir
    from contextlib import ExitStack

    dt = mybir.dt
    AF = mybir.ActivationFunctionType
    OP = mybir.AluOpType
    AX = mybir.AxisListType
    f32, f16, i16, i32 = dt.float32, dt.float16, dt.int16, dt.int32
    USE_F32R = bool(int(os.environ.get("BASS_F32R", "1")))
    f32r = dt.float32r if USE_F32R else dt.float32
    def r32(ap):
        return ap.bitcast(f32r)

    nc = bacc.Bacc("TRN2", target_bir_lowering=False, debug=False)

    def din(name, shape, dtyp=f32):
        return nc.dram_tensor(name, shape, dtyp, kind="ExternalInput").ap()

    xbT = din("xbT", (64, 1024))
    xsT = din("xsT", (64, TS))
    inpT = din("inpT", (65, TS))
    W1Ta = din("W1Ta", (65, 128))
    W1Tb = din("W1Tb", (65, 128))
    bp1c = din("bp1c", (256, 1))
    W2Ta = din("W2Ta", (128, 16))
    W2Tb = din("W2Tb", (128, 16))
    bp2c = din("bp2c", (16, 1))
    WkTs = din("WkTs", (64, 512))
    WqT = din("WqT", (64, 512))
    WvT = din("WvT", (64, 512))
    WuT = din("WuT", (512, 64))
    buR = din("buR", (1, 64))
    tposP = din("tposP", (TS, 1))
    gidxF = din("gidxF", (TS, 32))
    roffM = din("roffM", (TS, 32))
    Ssel = din("Ssel", (32, 128))
    DBG = bool(int(os.environ.get("BASS_KERNEL_DEBUG", "0")))
    idF16 = din("idF16", (128, 128), f16)
    idF32 = din("idF32", (128, 128), f32)
    outD = nc.dram_tensor("out", (TS, 64), f32, kind="ExternalOutput").ap()
    dbg = {}
    if DBG:
        def dout(name, shape, dtyp):
            dbg[name] = nc.dram_tensor(name, shape, dtyp, kind="ExternalOutput").ap()
        dout("d_idx", (TS, 80), f32)
        dout("d_cntL", (TS, 80), f16)
        dout("d_cntR", (TS, 80), f16)
        dout("d_wts", (TS, 80), f32)
        dout("d_wd", (TS, 1024), f16)
        dout("d_md", (TS, 1024), f16)
        dout("d_dt", (128, 2048), f16)
        dout("d_eb", (128, 2048), f16)
        dout("d_psO", (65, 2048), f32)
        dout("d_invZ", (8, TS), f32)

    def vap(t, dims, off=0):
        a = t[:] if not isinstance(t, bass.AP) else t
        return bass.AP(tensor=a.tensor, offset=a.offset + off,
                       ap=[list(a.ap[0])] + [list(d) for d in dims])

    with tile.TileContext(nc) as tc, ExitStack() as ctx:
        P = ctx.enter_context(tc.tile_pool(name="persist", bufs=1))
        WK = ctx.enter_context(tc.tile_pool(name="work", bufs=2))
        W1 = ctx.enter_context(tc.tile_pool(name="work1", bufs=1))
        PS = ctx.enter_context(tc.tile_pool(name="psum", bufs=2, space="PSUM"))
        PSD = ctx.enter_context(tc.tile_pool(name="psumd", bufs=2, space="PSUM"))
        PSO = ctx.enter_context(tc.tile_pool(name="psumO", bufs=1, space="PSUM"))

        # ---------------- const loads ----------------
        def load(name, ap_in, shape, dtyp=f32, tag=None):
            t = P.tile(list(shape), dtyp, tag=tag or name)
            nc.sync.dma_start(out=t[:], in_=ap_in)
            return t

        inpT_sb = load("inpT_sb", inpT, (65, TS))
        w1a = load("w1a", W1Ta, (65, 128))
        w1b = load("w1b", W1Tb, (65, 128))
        bp1a = load("bp1a", bp1c[0:128, :], (128, 1))
        bp1b = load("bp1b", bp1c[128:256, :], (128, 1))
        w2a = load("w2a", W2Ta, (128, 16))
        w2b = load("w2b", W2Tb, (128, 16))
        bp2_sb = load("bp2_sb", bp2c, (16, 1))
        idf16 = load("idf16", idF16, (128, 128), f16)
        idf32 = load("idf32", idF32, (128, 128), f32)
        tp_sb = [load(f"tp{tt}", tposP[tt * 128:(tt + 1) * 128, :], (128, 1))
                 for tt in range(2)]
        gi_sb = [load(f"gi{tt}", gidxF[tt * 128:(tt + 1) * 128, :], (128, 32))
                 for tt in range(2)]
        ro_sb = [load(f"ro{tt}", roffM[tt * 128:(tt + 1) * 128, :], (128, 32))
                 for tt in range(2)]
        xsT_f = load("xsT_f", xsT, (64, TS))
        xbT_f = load("xbT_f", xbT, (64, 1024))
        wk_f = load("wk_f", WkTs, (64, 512))
        wq_f = load("wq_f", WqT, (64, 512))
        wv_f = load("wv_f", WvT, (64, 512))
        def conv_r(nm, tf, shape, eng):
            t = P.tile(list(shape), f32r, tag=nm, name=nm)
            if eng == "a":
                nc.scalar.copy(t[:], tf[:])
            else:
                nc.vector.tensor_copy(t[:], tf[:])
            return t
        xsT_sb = conv_r("xsT_sb", xsT_f, (64, TS), "v")
        xbT_sb = conv_r("xbT_sb", xbT_f, (64, 1024), "a")
        wk_sb = conv_r("wk_sb", wk_f, (64, 512), "v")
        wq_sb = conv_r("wq_sb", wq_f, (64, 512), "a")
        wv_sb = conv_r("wv_sb", wv_f, (64, 512), "v")
        wu_sb = [load(f"wu{pc}", WuT[pc * 128:(pc + 1) * 128, :], (128, 64))
                 for pc in range(4)]
        bu_sb = load("bu_sb", buR, (1, 64))
        ssel_sb = [load(f"ssel{hp}", Ssel[hp * 8:(hp + 1) * 8, :], (8, 128))
                   for hp in range(4)]
        c1023 = P.tile([128, 1], f32, tag="c1023", name="c1023")
        nc.vector.memset(c1023[:], 1023.0)
        ones1 = P.tile([1, 128], f32, tag="ones1", name="ones1")
        nc.vector.memset(ones1[:], 1.0)
        c1 = P.tile([128, 1], f32, tag="c1", name="c1")
        nc.vector.memset(c1[:], 1.0)
        c2 = P.tile([128, 1], f32, tag="c2", name="c2")
        nc.vector.memset(c2[:], 2.0)
        cm1 = P.tile([128, 1], f32, tag="cm1", name="cm1")
        nc.vector.memset(cm1[:], -1.0)
        cm8 = P.tile([128, 1], f32, tag="cm8", name="cm8")
        nc.vector.memset(cm8[:], -8.0)
        cmexp = P.tile([128, 1], f32, tag="cmexp", name="cmexp")
        nc.vector.memset(cmexp[:], -float(np.exp(-8.0)))
        zl = P.tile([1, 65], f32, tag="zl", name="zl")
        nc.vector.memset(zl[:], 0.0)
        zr = P.tile([1, 512], f32, tag="zr", name="zr")
        nc.vector.memset(zr[:], 0.0)

        # ---------------- hypernet ----------------
        r1 = [P.tile([128, TS], f32, tag=f"r1{i}", name=f"r1{i}") for i in range(2)]
        for i, (w1, bp1_) in enumerate([(w1a, bp1a), (w1b, bp1b)]):
            p1 = PS.tile([128, TS], f32, tag="ps", name="p1")
            nc.tensor.matmul(p1[:], w1[:], inpT_sb[:], start=True, stop=True)
            nc.scalar.activation(r1[i][:], p1[:], AF.Relu, bias=bp1_[:])
        p2 = PS.tile([16, TS], f32, tag="ps", name="p2")
        nc.tensor.matmul(p2[:], w2a[:], r1[0][:], start=True, stop=False)
        nc.tensor.matmul(p2[:], w2b[:], r1[1][:], start=False, stop=True)
        p2T_sb = P.tile([16, TS], f32, tag="p2T", name="p2T")
        nc.scalar.activation(p2T_sb[:], p2[:], AF.Identity, bias=bp2_sb[:])
        p2_sb = [P.tile([128, 16], f32, tag=f"p2_{tt}", name=f"p2_{tt}") for tt in range(2)]
        for tt in range(2):
            pt = PS.tile([128, 16], f32, tag="ps", name="pt")
            nc.tensor.transpose(pt[:], p2T_sb[:, tt * 128:(tt + 1) * 128],
                                idf32[0:16, 0:16])
            nc.scalar.copy(p2_sb[tt][:], pt[:])

        # ---------------- projections ----------------
        # K^T (512,1024) as 4 chunks (128,1024); rows h*64+e, scaled 1/8 via WkTs
        kt = [P.tile([128, 1024], f32r, tag=f"kt{mc}", name=f"kt{mc}") for mc in range(4)]
        qt = [P.tile([128, TS], f32r, tag=f"qt{mc}", name=f"qt{mc}") for mc in range(4)]
        for mc in range(4):
            for half in range(2):
                pk = PS.tile([128, 512], f32, tag="ps", name="pk")
                nc.tensor.matmul(pk[:], r32(wk_sb[:, mc * 128:(mc + 1) * 128]),
                                 r32(xbT_sb[:, half * 512:(half + 1) * 512]),
                                 start=True, stop=True)
                if half == 0:
                    nc.scalar.copy(kt[mc][:, half * 512:(half + 1) * 512], pk[:])
                else:
                    nc.vector.tensor_copy(kt[mc][:, half * 512:(half + 1) * 512], pk[:])
            pq = PS.tile([128, TS], f32, tag="ps", name="pq")
            nc.tensor.matmul(pq[:], r32(wq_sb[:, mc * 128:(mc + 1) * 128]),
                             r32(xsT_sb[:]), start=True, stop=True)
            (nc.scalar.copy if mc % 2 == 0 else nc.vector.tensor_copy)(qt[mc][:], pq[:])

        # V_aug (j,520) f32 per j-chunk: cols h*65+[0..63] = V, col h*65+64 = 1
        vaug = [P.tile([128, 520], f16, tag=f"va{jc}", name=f"va{jc}") for jc in range(8)]
        for jc in range(8):
            pv = PS.tile([128, 512], f32, tag="ps", name="pv")
            nc.tensor.matmul(pv[:], r32(xbT_sb[:, jc * 128:(jc + 1) * 128]),
                             r32(wv_sb[:]), start=True, stop=True)
            va3 = vaug[jc].rearrange("p (h c) -> p h c", h=8)
            if jc % 2 == 0:
                nc.scalar.copy(va3[:, :, 0:64], pv[:].rearrange("p (h c) -> p h c", h=8))
            else:
                nc.vector.tensor_copy(va3[:, :, 0:64], pv[:].rearrange("p (h c) -> p h c", h=8))
            nc.vector.memset(va3[:, :, 64:65], 1.0)

        # ---------------- per t-tile sparse stage ----------------
        wd = [P.tile([128, 1024], f16, tag=f"wd{tt}", name=f"wd{tt}") for tt in range(2)]
        md = [P.tile([128, 1024], f16, tag=f"md{tt}", name=f"md{tt}") for tt in range(2)]
        for tt in range(2):
            tp = tp_sb[tt]
            # means / sigmas
            sp = WK.tile([128, 8], f32, tag="sp", name="sp")
            nc.scalar.activation(sp[:], p2_sb[tt][:, 0:8], AF.Exp)
            nc.scalar.activation(sp[:], sp[:], AF.Ln, bias=c1[:])
            means = WK.tile([128, 8], f32, tag="means", name="means")
            nc.vector.scalar_tensor_tensor(means[:], sp[:], -3.0,
                                           vap(tp, [[0, 8]]), OP.mult, OP.add)
            nc.vector.scalar_tensor_tensor(means[:], means[:], 0.0,
                                           vap(c1023, [[0, 8]]), OP.max, OP.min)
            fli = WK.tile([128, 8], i32, tag="fli", name="fli")
            nc.vector.tensor_copy(fli[:], means[:])
            flr = WK.tile([128, 8], f32, tag="flr", name="flr")
            nc.vector.tensor_copy(flr[:], fli[:])
            # robust floor under either cast-rounding mode: r - (r > x)
            fgt = WK.tile([128, 8], f32, tag="fgt", name="fgt")
            nc.vector.tensor_tensor(fgt[:], flr[:], means[:], OP.is_gt)
            fl = WK.tile([128, 8], f32, tag="fl", name="fl")
            nc.vector.tensor_tensor(fl[:], flr[:], fgt[:], OP.subtract)
            sp2 = WK.tile([128, 8], f32, tag="sp2", name="sp2")
            nc.scalar.activation(sp2[:], p2_sb[tt][:, 8:16], AF.Exp, bias=c2[:])
            nc.scalar.activation(sp2[:], sp2[:], AF.Ln, bias=c1[:])
            sig = WK.tile([128, 8], f32, tag="sig", name="sig")
            nc.vector.tensor_scalar(sig[:], sp2[:], 0.05, 102.4, OP.add, OP.mult)
            invsig = WK.tile([128, 8], f32, tag="invsig", name="invsig")
            nc.vector.reciprocal(invsig[:], sig[:])

            # candidate indices (128, 80) = 8 groups of [n0 n1 g g g g r r r r]
            idxF = WK.tile([128, 8, 10], f32, tag="idxF", name="idxF")
            nc.vector.tensor_copy(idxF[:, :, 0:1], vap(fl, [[1, 8], [0, 1]]))
            nc.vector.scalar_tensor_tensor(idxF[:, :, 1:2], vap(fl, [[1, 8], [0, 1]]), 1.0,
                                           vap(c1023, [[0, 8], [0, 1]]), OP.add, OP.min)
            nc.vector.tensor_copy(idxF[:, :, 2:6],
                                  gi_sb[tt][:].rearrange("p (k g) -> p k g", k=8))
            rtmp = WK.tile([128, 8, 4], f32, tag="rtmp", name="rtmp")
            nc.vector.tensor_tensor(rtmp[:], ro_sb[tt][:].rearrange("p (k g) -> p k g", k=8),
                                    vap(fl, [[1, 8], [0, 4]]), OP.add)
            nc.vector.scalar_tensor_tensor(idxF[:, :, 6:10], rtmp[:], 0.0,
                                           vap(c1023, [[0, 8], [0, 4]]), OP.max, OP.min)
            idxf = idxF[:].rearrange("p k g -> p (k g)")

            causal = WK.tile([128, VS], f32, tag="causal", name="causal")
            nc.vector.tensor_scalar(causal[:], idxf, tp[:], None, OP.is_gt)

            # pairwise-equality counts cntL / cntR over shift distances 1..79
            idxA = W1.tile([128, 80], f16, tag="idxA", name="idxA")
            nc.vector.tensor_copy(idxA[:], idxf)
            idxB = W1.tile([128, 160], f16, tag="idxB", name="idxB")
            nc.vector.memset(idxB[:, 0:80], -1.0)
            nc.vector.tensor_copy(idxB[:, 80:160], idxf)
            idxBo = W1.tile([128, 160], f16, tag="idxBo", name="idxBo")
            nc.vector.memset(idxBo[:, 159:160], -3.0)
            nc.vector.tensor_copy(idxBo[:, 0:159], idxB[:, 1:160])

            cbuf = W1.tile([128, 80 * 80], f16, tag="cbuf", name="cbuf")
            nc.vector.memset(cbuf[:, 0:80], 0.0)
            # even d = 2,4,..,78 (39 planes): in1 row start 80-d (even)
            nc.vector.tensor_tensor(
                vap(cbuf, [[160, 39], [1, 80]], off=160),
                vap(idxA, [[0, 39], [1, 80]]),
                vap(idxB, [[-2, 39], [1, 80]], off=78), OP.is_equal)
            # odd d = 1,3,..,79 (40 planes): idxBo[i] = idxB[i+1], start 80-d-1 (even)
            nc.vector.tensor_tensor(
                vap(cbuf, [[160, 40], [1, 80]], off=80),
                vap(idxA, [[0, 40], [1, 80]]),
                vap(idxBo, [[-2, 40], [1, 80]], off=78), OP.is_equal)

            def tree(first_level_in0, first_level_in1, tag, eng):
                lv = W1.tile([128, 40 * 80], f16, tag=f"{tag}40", name=f"{tag}40")
                eng.tensor_tensor(lv[:], first_level_in0, first_level_in1, OP.add)
                sizes = [20, 10, 5]
                for s in sizes:
                    nxt = W1.tile([128, s * 80], f16, tag=f"{tag}{s}", name=f"{tag}{s}")
                    eng.tensor_tensor(
                        nxt[:], vap(lv, [[160, s], [1, 80]]),
                        vap(lv, [[160, s], [1, 80]], off=80), OP.add)
                    lv = nxt
                # 5 planes -> (0+1)+(2+3) then +4
                s2 = W1.tile([128, 2 * 80], f16, tag=f"{tag}2", name=f"{tag}2")
                eng.tensor_tensor(s2[:], vap(lv, [[160, 2], [1, 80]]),
                                        vap(lv, [[160, 2], [1, 80]], off=80), OP.add)
                s1 = WK.tile([128, 80], f16, tag=f"{tag}1", name=f"{tag}1")
                eng.tensor_tensor(s1[:], s2[:, 0:80], s2[:, 80:160], OP.add)
                cnt = WK.tile([128, 80], f16, tag=f"{tag}c", name=f"{tag}c")
                eng.tensor_tensor(cnt[:], s1[:], vap(lv, [[1, 80]], off=320),
                                        OP.add)
                return cnt

            # L: plane d at offset 80*d ; pairs (2e, 2e+1), d=0 plane is zero
            cntL = tree(vap(cbuf, [[160, 40], [1, 80]]),
                        vap(cbuf, [[160, 40], [1, 80]], off=80), "tl", nc.vector)

            firstf = WK.tile([128, VS], f32, tag="firstf", name="firstf")
            nc.vector.tensor_scalar(firstf[:], cntL[:], 0.0, None, OP.is_equal)
            pmask = WK.tile([128, VS], f32, tag="pmask", name="pmask")
            nc.vector.tensor_tensor(pmask[:], firstf[:], causal[:], OP.is_gt)

            # props (v-major, k-minor) and mixture weights
            zd = W1.tile([128, VS, 8], f32, tag="zd", name="zd")
            nc.vector.tensor_tensor(zd[:], vap(idxF, [[1, 80], [0, 8]]),
                                    vap(means, [[0, 80], [1, 8]]), OP.subtract)
            nc.vector.tensor_tensor(zd[:], zd[:], vap(invsig, [[0, 80], [1, 8]]),
                                    OP.mult)
            sq = W1.tile([128, VS, 8], f32, tag="sq", name="sq")
            nc.scalar.activation(sq[:], zd[:], AF.Square)
            props = W1.tile([128, VS, 8], f32, tag="props", name="props")
            nc.scalar.activation(props[:], sq[:], AF.Exp, scale=-0.5)
            nc.vector.tensor_tensor(props[:], props[:], vap(pmask, [[1, 80], [0, 8]]),
                                    OP.mult)
            ssum = WK.tile([128, 8], f32, tag="ssum", name="ssum")
            nc.vector.tensor_reduce(ssum[:], vap(props, [[1, 8], [8, 80]]),
                                    AX.X, OP.add)
            invS = WK.tile([128, 8], f32, tag="invS", name="invS")
            nc.vector.reciprocal(invS[:], ssum[:])
            nc.vector.tensor_tensor(props[:], props[:], vap(invS, [[0, 80], [1, 8]]),
                                    OP.mult)
            wts = WK.tile([128, VS], f32, tag="wts", name="wts")
            nc.vector.tensor_reduce(wts[:], props[:], AX.X, OP.add)

            # scatter data / masked indices
            w16 = WK.tile([128, VS], f16, tag="w16", name="w16")
            nc.vector.tensor_copy(w16[:], wts[:])
            mult16 = WK.tile([128, VS], f16, tag="mult16", name="mult16")
            nc.vector.tensor_scalar(mult16[:], cntL[:], 1.0, None, OP.add)
            idm = WK.tile([128, VS], f32, tag="idm", name="idm")
            nc.vector.scalar_tensor_tensor(idm[:], idxf, 1.0, firstf[:],
                                           OP.add, OP.mult)
            nc.vector.tensor_scalar(idm[:], idm[:], -1.0, None, OP.add)
            idm16 = WK.tile([128, VS], i16, tag="idm16", name="idm16")
            nc.vector.tensor_copy(idm16[:], idm[:])
            idall16 = WK.tile([128, VS], i16, tag="idall16", name="idall16")
            nc.vector.tensor_copy(idall16[:], idxf)

            nc.gpsimd.local_scatter(wd[tt][:], w16[:], idm16[:], 128, 1024, VS)
            nc.gpsimd.local_scatter(md[tt][:], mult16[:], idall16[:], 128, 1024, VS)
            if DBG:
                sl = slice(tt * 128, (tt + 1) * 128)
                nc.sync.dma_start(out=dbg["d_idx"][sl, :], in_=idxf)
                nc.sync.dma_start(out=dbg["d_cntL"][sl, :], in_=cntL[:])
                nc.sync.dma_start(out=dbg["d_cntR"][sl, :], in_=mult16[:])
                nc.sync.dma_start(out=dbg["d_wts"][sl, :], in_=wts[:])
                nc.sync.dma_start(out=dbg["d_wd"][sl, :], in_=wd[tt][:])
                nc.sync.dma_start(out=dbg["d_md"][sl, :], in_=md[tt][:])

        # ---------------- transposes of Wd / Md ----------------
        wdT = [P.tile([128, TS], f16, tag=f"wdT{jc}", name=f"wdT{jc}") for jc in range(8)]
        mm1T = [P.tile([128, TS], f16, tag=f"mm1T{jc}", name=f"mm1T{jc}") for jc in range(8)]
        for jc in range(8):
            for tt in range(2):
                ptw = PS.tile([128, 128], f16, tag="ps", name="ptw")
                nc.tensor.transpose(ptw[:], wd[tt][:, jc * 128:(jc + 1) * 128],
                                    idf16[:])
                nc.scalar.copy(wdT[jc][:, tt * 128:(tt + 1) * 128], ptw[:])
                ptm = PS.tile([128, 128], f16, tag="ps", name="ptm")
                nc.tensor.transpose(ptm[:], md[tt][:, jc * 128:(jc + 1) * 128],
                                    idf16[:])
                nc.scalar.activation(mm1T[jc][:, tt * 128:(tt + 1) * 128], ptm[:],
                                     AF.Identity, bias=cmexp[:],
                                     scale=float(np.exp(-8.0)))

        # ---------------- dense phase: per j-chunk ----------------
        psO = PSO.tile([65, 8, TS], f32, tag="psO", name="psO")
        zsp = P.tile([8, TS], f32, tag="zsp", name="zsp")
        # each start=True clears its whole PSUM bank, so pre-clear each of the
        # 4 banks once (k=1 zero matmul over the full 512-f32 bank span) and
        # accumulate everything else with start=False.
        for m in range(4):
            nc.tensor.matmul(vap(psO, [[1, 512]], off=512 * m), zl[:], zr[:],
                             start=True, stop=False, skip_group_check=True)
        for jc in range(8):
            dt16 = WK.tile([128, 8, TS], f16, tag="dt16", name="dt16", bufs=8)
            for h in range(8):
                pd = PSD.tile([128, TS], f32, tag="pd", name="pd")
                nc.tensor.matmul(
                    pd[:],
                    r32(kt[h // 2][(h % 2) * 64:(h % 2) * 64 + 64, jc * 128:(jc + 1) * 128]),
                    r32(qt[h // 2][(h % 2) * 64:(h % 2) * 64 + 64, :]),
                    start=True, stop=True)
                nc.scalar.copy(dt16[:, h:h + 1, :], vap(pd, [[0, 1], [1, TS]]))
            lg16 = WK.tile([128, 8, TS], f16, tag="lg16", name="lg16")
            nc.vector.tensor_tensor(lg16[:], dt16[:],
                                    vap(wdT[jc], [[0, 8], [1, TS]]), OP.mult)
            ebig = WK.tile([128, 8, TS], f16, tag="ebig", name="ebig")
            nc.scalar.activation(ebig[:], lg16[:], AF.Exp, bias=cm8[:])
            if DBG and jc == 0:
                nc.sync.dma_start(out=dbg["d_dt"][:], in_=dt16[:])
                nc.sync.dma_start(out=dbg["d_eb"][:], in_=ebig[:])
            for h in range(8):
                va = vaug[jc][:, h * 65:h * 65 + 65]
                nc.tensor.matmul(psO[:, h, :], va, ebig[:, h, :],
                                 start=False, stop=False, skip_group_check=True)
                nc.tensor.matmul(psO[:, h, :], va, mm1T[jc][:],
                                 start=False, stop=(jc == 7), skip_group_check=True)


        # ---------------- normalize + output proj ----------------
        if DBG:
            dps = P.tile([65, 8, TS], f32, tag="dps", name="dps")
            nc.scalar.copy(dps[:], psO[:])
            nc.sync.dma_start(out=dbg["d_psO"][:], in_=dps[:].rearrange("p a b -> p (a b)"))
        invZf = P.tile([1, 8, TS], f32, tag="invZf", name="invZf")
        nc.vector.reciprocal(invZf[:], psO[64:65, :, :])
        invZ = P.tile([8, TS], f32, tag="invZ", name="invZ")
        nc.sync.dma_start(out=invZ[:], in_=invZf[:])
        if DBG:
            nc.sync.dma_start(out=dbg["d_invZ"][:], in_=invZ[:])
        packed = [P.tile([128, TS], f32, tag=f"packed{pc}", name=f"packed{pc}") for pc in range(4)]
        for hp in range(4):
            pb = PS.tile([128, TS], f32, tag="ps", name="pb")
            nc.tensor.matmul(pb[:], ssel_sb[hp][:], invZ[:],
                             start=True, stop=True)
            bc = WK.tile([128, TS], f32, tag="bc", name="bc")
            nc.scalar.copy(bc[:], pb[:])
            for i in range(2):
                h = 2 * hp + i
                nc.vector.tensor_tensor(packed[hp][i * 64:(i + 1) * 64, :],
                                        psO[0:64, h, :], bc[i * 64:(i + 1) * 64, :],
                                        OP.mult)
        for th in range(2):
            pf = PS.tile([128, 64], f32, tag="ps", name="pf")
            for pc in range(4):
                nc.tensor.matmul(pf[:], packed[pc][:, th * 128:(th + 1) * 128],
                                 wu_sb[pc][:], start=(pc == 0), stop=False)
            nc.tensor.matmul(pf[:], ones1[:], bu_sb[:],
                             start=False, stop=True)
            ofin = WK.tile([128, 64], f32, tag="ofin", name="ofin")
            nc.scalar.copy(ofin[:], pf[:])
            nc.sync.dma_start(out=outD[th * 128:(th + 1) * 128, :], in_=ofin[:])

    nc.compile()
    return nc


def _make_in_maps(inputs):
    x = np.asarray(inputs["x"], np.float32)
    Wk = np.asarray(inputs["Wk"], np.float32)
    Wq = np.asarray(inputs["Wq"], np.float32)
    Wv = np.asarray(inputs["Wv"], np.float32)
    Wu = np.asarray(inputs["Wu"], np.float32)
    bu = np.asarray(inputs["bu"], np.float32)
    Wp1 = np.asarray(inputs["Wp1"], np.float32)
    bp1 = np.asarray(inputs["bp1"], np.float32)
    Wp2 = np.asarray(inputs["Wp2"], np.float32)
    bp2 = np.asarray(inputs["bp2"], np.float32)

    gidx, roff = _host_constants()
    tpos = np.arange(T, dtype=np.float32)
    C = np.ascontiguousarray

    ssel = np.zeros((32, 128), np.float32)
    for hp in range(4):
        for r in range(8):
            for p in range(128):
                if r == 2 * hp + p // 64:
                    ssel[hp * 8 + r, p] = 1.0
    ident16 = np.eye(128, dtype=np.float16)

    shared = {
        "W1Ta": C(Wp1[0:128].T), "W1Tb": C(Wp1[128:256].T),
        "bp1c": C(bp1.reshape(256, 1)),
        "W2Ta": C(Wp2[:, 0:128].T), "W2Tb": C(Wp2[:, 128:256].T),
        "bp2c": C(bp2.reshape(16, 1)),
        "WkTs": C(Wk.T / 8.0), "WqT": C(Wq.T), "WvT": C(Wv.T),
        "WuT": C(Wu.T), "buR": C(bu.reshape(1, 64)),
        "Ssel": ssel, "idF16": ident16,
        "idF32": np.eye(128, dtype=np.float32),
    }
    in_maps = []
    for c in range(NC):
        b, t0 = c // 4, TS * (c % 4)
        ts = slice(t0, t0 + TS)
        m = dict(shared)
        m["xbT"] = C(x[b].T)
        m["xsT"] = C(x[b, ts].T)
        m["inpT"] = C(np.vstack([x[b, ts].T, (tpos[ts] / T)[None, :]]))
        m["tposP"] = C(tpos[ts].reshape(TS, 1))
        m["gidxF"] = C(gidx[b, ts].reshape(TS, 32).astype(np.float32))
        m["roffM"] = C((roff[b, ts].reshape(TS, 32) - REGION // 2).astype(np.float32))
        in_maps.append(m)
    return in_maps


def kernel(**inputs):
    from concourse import bass_utils
    if "nc" not in _prog_cache:
        _prog_cache["nc"] = build_program()
    nc = _prog_cache["nc"]
    in_maps = _make_in_maps(inputs)
    res = bass_utils.run_bass_kernel_spmd(
        nc, in_maps, core_ids=list(range(NC)),
        trace=bool(int(os.environ.get("BASS_KERNEL_TRACE", "0"))))
    _prog_cache["last_result"] = res
    out = np.zeros((B, T, E), np.float32)
    for c in range(NC):
        b, t0 = c // 4, TS * (c % 4)
        out[b, t0:t0 + TS] = res.results[c]["out"]
    return out
